# revision 2
# baseline (speedup 1.0000x reference)
"""GCN (2-layer) on 8 Trainium2 NeuronCores via Bass — single-NEFF design.

Math (norm = dinv[src]*dinv[dst] is separable; b-terms folded):
  g1 = (dinv*x) @ W1.T + dinv*b1                    per-core row shard
  S1[d] = sum_{(s,d) in E} g1[s] + g1[d]            (self-loop term analytic)
  g2 = relu(dinv^2 * S1)
  S2[d] = sum_{(s,d) in E} g2[s] + g2[d]
  out = log_softmax(dinv * (S2 @ W2.T + rvec * b2))

Everything runs in ONE NEFF per core:
  - phase A: dense matmul g1 = xsT @ W1 (x shipped as fp8, 6.4MB/core)
  - AllGather g1 across the 8 cores (bf16 halo exchange, on-device)
  - sparse aggregation via GPSIMD indirect_copy gathers + fp32 prefix-sum +
    boundary-difference segment sums (edge indices shipped as uint16 streams)
  - AllGather g2, second aggregation, head matmul + log_softmax on device

Host does only index preprocessing (bucket/sort edges, build uint16 gather
streams) and the final unshard. Wire traffic/call ~70MB vs ~380MB for the
3-NEFF host-gather design.

HW constraints baked in (probed on this axon terminal):
  - indirect_copy: src table <= 16KB/partition, <=1024 dst elems/instruction,
    4-byte or 2-byte dtypes (fp8 rejected), idx uint16 wrapped per 16
    partitions. GPSIMD loadable-library ops (dma_gather etc.) hard-crash;
    indirect_copy is base-ucode and works.
  - tables bf16 [128 x 12544] gathered via half-shard (6272-col) views.
"""
import os
import sys

for _p in ("/opt/trn_rl_repo", "/root/.axon_site/_ro/trn_rl_repo"):
    if os.path.isdir(_p) and _p not in sys.path:
        sys.path.insert(0, _p)

import ml_dtypes
import numpy as np

from concourse import bass, bacc, mybir
from concourse import tile
from concourse import bass2jax

N = 100000
F_IN = 512
HID = 16
CLS = 40
NCORES = 8
NP = N // NCORES            # 12500 nodes per shard
NPAD = 12544                # 98*128
HALF = 6272                 # half-shard table view (bf16 -> 12544B < 16KB)
TN = 1792                   # dst nodes per aggregation tile
NT = NPAD // TN             # 7 tiles
CH = 448                    # column chunk (PSUM-sized)
NCH = NPAD // CH            # 28
KPT = TN // CH              # 4 chunks per tile
MPT = TN // 128             # 14 head blocks per tile
NB = 1824                   # boundary gather cols (1793 padded to 114*16)
SBH = NB // 16              # 114 idx cols per (tile, half); even (4B idx align)

FP32 = mybir.dt.float32
BF16 = mybir.dt.bfloat16
FP8 = mybir.dt.float8e4
U16 = mybir.dt.uint16
NPBF = ml_dtypes.bfloat16
NPF8 = ml_dtypes.float8_e4m3

_EXEC_NS = {"total": 0.0, "have": False, "walls": []}
_NEFF_CACHE = {}
_JIT_CACHE = {}


# ---------------------------------------------------------------------------
# NEFF builder
# ---------------------------------------------------------------------------
def build_neff(Ws):
    """Ws: tuple of NT tuples (W_h0, W_h1) — gather stream widths, mult of 16,
    <= 4096 (fp32 cumsum must stay under the 16KB indirect_copy src limit)."""
    AF = mybir.ActivationFunctionType
    OP = mybir.AluOpType
    AX = mybir.AxisListType

    SM = sum(w // 16 for tw in Ws for w in tw)   # total msg idx cols
    SBT = NT * 2 * SBH                            # total boundary idx cols

    nc = bacc.Bacc("TRN2")
    xT = nc.declare_dram_parameter("xT", [F_IN, NPAD], FP8, isOutput=False)
    w1t = nc.declare_dram_parameter("w1t", [128, 4, HID], BF16, isOutput=False)
    b1r = nc.declare_dram_parameter("b1r", [1, HID], BF16, isOutput=False)
    dvr = nc.declare_dram_parameter("dvr", [NCH, CH], BF16, isOutput=False)
    dv2 = nc.declare_dram_parameter("dv2", [NCH, CH], FP32, isOutput=False)
    rvr = nc.declare_dram_parameter("rvr", [NCH, CH], FP32, isOutput=False)
    dsc = nc.declare_dram_parameter("dsc", [128, NPAD // 128], FP32, isOutput=False)
    w2b = nc.declare_dram_parameter("w2b", [HID + 1, CLS], FP32, isOutput=False)
    g8m = nc.declare_dram_parameter("g8m", [128, 16], FP32, isOutput=False)
    mix = nc.declare_dram_parameter("mix", [128, SM], U16, isOutput=False)
    bix = nc.declare_dram_parameter("bix", [128, SBT], U16, isOutput=False)
    oout = nc.declare_dram_parameter("oout", [NPAD, CLS], BF16, isOutput=True)

    with tile.TileContext(nc) as tc:
        with (
            tc.tile_pool(name="const", bufs=1) as constp,
            tc.tile_pool(name="gtab", bufs=1) as gtabp,      # bf16 gather table
            tc.tile_pool(name="gloc", bufs=1) as glocp,      # per-core g1/g2
            tc.tile_pool(name="xsl", bufs=2) as xslp,
            tc.tile_pool(name="row", bufs=2) as rowp,        # [1, CH] scale rows
            tc.tile_pool(name="mg", bufs=1) as mgp,
            tc.tile_pool(name="cs", bufs=1) as csp,
            tc.tile_pool(name="gb", bufs=2) as gbp,
            tc.tile_pool(name="dd", bufs=3) as ddp,
            tc.tile_pool(name="post", bufs=2) as postp,
            tc.tile_pool(name="head", bufs=1) as headp,
            tc.tile_pool(name="sml", bufs=2) as smlp,
            tc.tile_pool(name="ps16", bufs=2, space="PSUM") as ps16,
            tc.tile_pool(name="psbc", bufs=2, space="PSUM") as psbc,
            tc.tile_pool(name="psout", bufs=2, space="PSUM") as psout,
            tc.tile_pool(name="dram", bufs=2, space="DRAM") as dramp,
        ):
            # ---- constants ----
            w1_sb = constp.tile([128, 4, HID], BF16)
            nc.sync.dma_start(out=w1_sb[:], in_=w1t[:])
            b1_sb = constp.tile([1, HID], BF16)
            nc.sync.dma_start(out=b1_sb[:], in_=b1r[:])
            dsc_sb = constp.tile([128, NPAD // 128], FP32)
            nc.sync.dma_start(out=dsc_sb[:], in_=dsc[:])
            w2b_sb = constp.tile([HID + 1, CLS], FP32)
            nc.sync.dma_start(out=w2b_sb[:], in_=w2b[:])
            g8_sb = constp.tile([128, 16], FP32)
            nc.sync.dma_start(out=g8_sb[:], in_=g8m[:])
            mix_sb = constp.tile([128, SM], U16)
            nc.sync.dma_start(out=mix_sb[:], in_=mix[:])
            bix_sb = constp.tile([128, SBT], U16)
            nc.sync.dma_start(out=bix_sb[:], in_=bix[:])
            ones16 = constp.tile([1, 16], FP32)
            nc.vector.memset(ones16[:], 1.0)

            xTv = xT.ap().rearrange("(kc p) n -> p kc n", p=128)

            # ---- phase A: g1loc = (dinv*x) @ W1.T + dinv*b1  (bf16 out) ----
            g1loc = glocp.tile([HID, NPAD], BF16, tag="g1")
            for j in range(NCH):
                c0 = j * CH
                xsb = xslp.tile([128, 4, CH], FP8, tag="xsb")
                nc.sync.dma_start(out=xsb[:], in_=xTv[:, :, c0:c0 + CH])
                dvrc = rowp.tile([1, CH], BF16, tag="dvrc")
                nc.sync.dma_start(out=dvrc[:], in_=dvr[j:j + 1, :])
                ps = ps16.tile([HID, CH], FP32)
                for kc in range(4):
                    nc.tensor.matmul(
                        ps[:], w1_sb[:, kc, :], xsb[:, kc, :],
                        start=(kc == 0), stop=False,
                    )
                nc.tensor.matmul(
                    ps[:], b1_sb[:], dvrc[:], start=False, stop=True,
                )
                nc.scalar.activation(g1loc[:, c0:c0 + CH], ps[:], AF.Copy)

            # ---- halo exchange + aggregation, layer 1 then layer 2 ----
            def allgather(gl):
                agi = dramp.tile([HID, NPAD], BF16)
                ago = dramp.tile([128, NPAD], BF16)
                nc.gpsimd.dma_start(agi[:], gl[:])
                nc.gpsimd.collective_compute(
                    "AllGather", OP.bypass,
                    replica_groups=[list(range(NCORES))],
                    ins=[agi.opt()], outs=[ago.opt()],
                )
                tbl = gtabp.tile([128, NPAD], BF16)
                nc.sync.dma_start(out=tbl[:], in_=ago[:])
                return tbl

            def seg_sums(tbl, tau):
                """Segment-sum both halves of tile tau from gather table tbl.
                Returns Dt [128, TN] f32 (per-(feature,group) partial sums)."""
                Ds = []
                soff = sum(w // 16 for tw in Ws[:tau] for w in tw)
                for h in (0, 1):
                    W = Ws[tau][h]
                    tview = tbl[:, HALF * h: HALF * h + HALF]
                    mg = mgp.tile([128, 4096], BF16, tag="mg")
                    for j in range(0, W, 1024):
                        w = min(1024, W - j)
                        nc.gpsimd.indirect_copy(
                            mg[:, j:j + w], tview,
                            mix_sb[:, soff + j // 16: soff + (j + w) // 16],
                            i_know_ap_gather_is_preferred=True,
                        )
                    cs = csp.tile([128, 4096], FP32, tag="cs")
                    nc.vector.tensor_tensor_scan(
                        cs[:, 0:W], mg[:, 0:W], mg[:, 0:W], 0.0,
                        OP.add, OP.bypass,
                    )
                    boff = (tau * 2 + h) * SBH
                    G = gbp.tile([128, NB], FP32, tag="G")
                    nc.gpsimd.indirect_copy(
                        G[:, 0:1024], cs[:, 0:W], bix_sb[:, boff:boff + 64],
                        i_know_ap_gather_is_preferred=True,
                    )
                    nc.gpsimd.indirect_copy(
                        G[:, 1024:NB], cs[:, 0:W], bix_sb[:, boff + 64:boff + SBH],
                        i_know_ap_gather_is_preferred=True,
                    )
                    D = ddp.tile([128, TN], FP32, tag="D")
                    nc.vector.tensor_tensor(
                        D[:], G[:, 1:TN + 1], G[:, 0:TN], OP.subtract
                    )
                    Ds.append(D)
                    soff += W // 16
                Dt = ddp.tile([128, TN], FP32, tag="D")
                nc.vector.tensor_tensor(Dt[:], Ds[0][:], Ds[1][:], OP.add)
                return Dt

            tbl1 = allgather(g1loc)
            g2loc = glocp.tile([HID, NPAD], BF16, tag="g2")
            for tau in range(NT):
                Dt = seg_sums(tbl1, tau)
                for k in range(KPT):
                    jc = tau * KPT + k
                    c0 = jc * CH
                    ps = ps16.tile([HID, CH], FP32)
                    nc.tensor.matmul(
                        ps[:], g8_sb[:], Dt[:, k * CH:(k + 1) * CH],
                        start=True, stop=True,
                    )
                    dvc = rowp.tile([1, CH], FP32, tag="dvc")
                    nc.sync.dma_start(out=dvc[:], in_=dv2[jc:jc + 1, :])
                    pb = psbc.tile([HID, CH], FP32)
                    nc.tensor.matmul(
                        pb[:], ones16[:], dvc[:], start=True, stop=True,
                    )
                    t1 = postp.tile([HID, CH], FP32, tag="t1")
                    nc.vector.tensor_tensor(
                        t1[:], ps[:], g1loc[:, c0:c0 + CH], OP.add
                    )
                    t2 = postp.tile([HID, CH], FP32, tag="t2")
                    nc.vector.tensor_tensor(t2[:], t1[:], pb[:], OP.mult)
                    nc.scalar.activation(g2loc[:, c0:c0 + CH], t2[:], AF.Relu)

            tbl2 = allgather(g2loc)
            ooutv = oout.ap().rearrange("(t p) c -> p t c", p=128)
            for tau in range(NT):
                Dt = seg_sums(tbl2, tau)
                ag2r = headp.tile([HID + 1, TN], FP32, tag="agr")
                for k in range(KPT):
                    jc = tau * KPT + k
                    c0 = jc * CH
                    ps = ps16.tile([HID, CH], FP32)
                    nc.tensor.matmul(
                        ps[:], g8_sb[:], Dt[:, k * CH:(k + 1) * CH],
                        start=True, stop=True,
                    )
                    nc.vector.tensor_tensor(
                        ag2r[0:HID, k * CH:(k + 1) * CH],
                        ps[:], g2loc[:, c0:c0 + CH], OP.add,
                    )
                nc.sync.dma_start(
                    out=ag2r[HID:HID + 1, :],
                    in_=rvr[tau * KPT:(tau + 1) * KPT, :].rearrange(
                        "(o a) b -> o (a b)", o=1
                    ),
                )
                osb = headp.tile([128, MPT, CLS], FP32, tag="osb")
                for m in range(MPT):
                    gt = tau * MPT + m
                    po = psout.tile([128, CLS], FP32)
                    nc.tensor.matmul(
                        po[:], ag2r[:, m * 128:(m + 1) * 128], w2b_sb[:],
                        start=True, stop=True,
                    )
                    nc.scalar.activation(
                        osb[:, m, :], po[:], AF.Copy, scale=dsc_sb[:, gt:gt + 1]
                    )
                # log_softmax over classes
                nm = smlp.tile([128, MPT, 1], FP32, tag="nm")
                nc.vector.tensor_reduce(
                    nm[:, :, 0], osb[:], AX.X, OP.max, negate=True
                )
                sub = headp.tile([128, MPT, CLS], FP32, tag="sub")
                b0, b1_ = bass.broadcast_tensor_aps(osb[:], nm[:, :, 0:1])
                nc.vector.tensor_tensor(sub[:], b0, b1_, OP.add)
                ex = headp.tile([128, MPT, CLS], FP32, tag="ex")
                nc.scalar.activation(ex[:], sub[:], AF.Exp)
                ss = smlp.tile([128, MPT, 1], FP32, tag="ss")
                nc.vector.tensor_reduce(ss[:, :, 0], ex[:], AX.X, OP.add)
                ln = smlp.tile([128, MPT, 1], FP32, tag="ln")
                nc.scalar.activation(ln[:, :, 0], ss[:, :, 0], AF.Ln)
                ob = headp.tile([128, MPT, CLS], BF16, tag="ob")
                b2_, b3_ = bass.broadcast_tensor_aps(sub[:], ln[:, :, 0:1])
                nc.vector.tensor_tensor(ob[:], b2_, b3_, OP.subtract)
                nc.sync.dma_start(
                    out=ooutv[:, tau * MPT:(tau + 1) * MPT, :], in_=ob[:]
                )
    nc.finalize()
    return nc


# ---------------------------------------------------------------------------
# cached-jit SPMD runner (mirrors bass2jax.run_bass_via_pjrt, axon path)
# ---------------------------------------------------------------------------
def _get_jitted(nc):
    key = id(nc)
    if key in _JIT_CACHE:
        return _JIT_CACHE[key]
    import jax
    from jax.experimental.shard_map import shard_map
    from jax.sharding import Mesh, PartitionSpec

    bass2jax.install_neuronx_cc_hook()
    assert not nc.dbg_callbacks
    dbg_name = nc.dbg_addr.name if nc.dbg_addr is not None else None
    part_name = (
        nc.partition_id_tensor.name if nc.partition_id_tensor else None
    )

    in_names, out_names, out_avals, zero_shapes = [], [], [], []
    for alloc in nc.m.functions[0].allocations:
        if not isinstance(alloc, mybir.MemoryLocationSet):
            continue
        name = alloc.memorylocations[0].name
        if alloc.kind == "ExternalInput":
            if name != part_name:
                in_names.append(name)
        elif alloc.kind == "ExternalOutput":
            out_names.append(name)
            shape = tuple(alloc.tensor_shape)
            dtype = mybir.dt.np(alloc.dtype)
            out_avals.append(jax.core.ShapedArray(shape, dtype))
            zero_shapes.append((shape, dtype))
    n_params = len(in_names)
    all_names = list(in_names) + out_names
    if part_name is not None:
        all_names.append(part_name)

    def _body(*args):
        operands = list(args)
        if part_name is not None:
            operands.append(bass2jax.partition_id_tensor())
        outs = bass2jax._bass_exec_p.bind(
            *operands,
            out_avals=tuple(out_avals),
            in_names=tuple(all_names),
            out_names=tuple(out_names),
            lowering_input_output_aliases=(),
            sim_require_finite=True,
            sim_require_nnan=True,
            nc=nc,
        )
        return tuple(outs)

    devices = jax.devices()[:NCORES]
    mesh = Mesh(np.asarray(devices), ("core",))
    specs = (PartitionSpec("core"),) * (n_params + len(out_names))
    sharded = jax.jit(
        shard_map(
            _body, mesh=mesh, in_specs=specs,
            out_specs=(PartitionSpec("core"),) * len(out_names),
            check_rep=False,
        ),
        keep_unused=True,
    )
    # device-resident zero operands for the output slots: uploaded once and
    # reused every call (the NEFF writes every output element, so the
    # uninitialized PJRT result buffers don't need the zero-donation dance)
    from jax.sharding import NamedSharding
    shd = NamedSharding(mesh, PartitionSpec("core"))
    dev_zeros = [
        jax.device_put(np.zeros((NCORES * s[0], *s[1:]), dt), shd)
        for s, dt in zero_shapes
    ]
    _JIT_CACHE[key] = (
        sharded, in_names, out_names, out_avals, dev_zeros, dbg_name
    )
    return _JIT_CACHE[key]


def _run_neff(nc, glob_map):
    """glob_map: name -> pre-concatenated [NCORES*d0, ...] array."""
    import time as _time
    sharded, in_names, out_names, out_avals, dev_zeros, dbg_name = (
        _get_jitted(nc)
    )
    if dbg_name is not None:
        glob_map = dict(glob_map)
        glob_map[dbg_name] = np.zeros((NCORES, 2), np.uint32)
    t0 = _time.perf_counter()
    out_arrs = sharded(*[glob_map[name] for name in in_names], *dev_zeros)
    results = [
        {
            name: np.asarray(out_arrs[i]).reshape(
                NCORES, *out_avals[i].shape
            )[c]
            for i, name in enumerate(out_names)
        }
        for c in range(NCORES)
    ]
    _EXEC_NS["walls"].append(_time.perf_counter() - t0)
    return results


# ---------------------------------------------------------------------------
# host-side index preprocessing + kernel entry
# ---------------------------------------------------------------------------
def kernel(x, edge_index, W1, b1, W2, b2):
    _EXEC_NS["walls"] = []
    x = np.asarray(x, np.float32)
    ei = np.asarray(edge_index, np.int64)
    W1 = np.asarray(W1, np.float32)
    b1 = np.asarray(b1, np.float32)
    W2 = np.asarray(W2, np.float32)
    b2 = np.asarray(b2, np.float32)
    n = x.shape[0]

    loops = np.arange(n, dtype=np.int64)
    srcl = np.concatenate([ei[0], loops])
    dstl = np.concatenate([ei[1], loops])
    deg = np.bincount(srcl, minlength=n).astype(np.float32)
    dinv = deg ** -0.5
    rvec = np.bincount(dstl, weights=dinv[srcl], minlength=n).astype(np.float32)

    src = ei[0]
    dst = ei[1]

    # ---- per-core edge streams -------------------------------------------
    NBKT = NT * 2 * NCORES
    cores = []
    Wreq = np.zeros((NT, 2), np.int64)
    for c in range(NCORES):
        lo, hi = c * NP, (c + 1) * NP
        m = (dst >= lo) & (dst < hi)
        s = src[m]
        dl = (dst[m] - lo).astype(np.int64)
        g = s // NP
        nl = s - g * NP
        h = nl // HALF
        il = (nl - h * HALF).astype(np.uint16)
        tau = dl // TN
        bucket = ((tau * 2 + h) * NCORES + g).astype(np.int64)
        order = np.lexsort((dl, bucket))
        sb = bucket[order]
        si = il[order]
        sd = dl[order]
        bc = np.bincount(sb, minlength=NBKT)
        boff = np.concatenate([[0], np.cumsum(bc)])
        rank = np.arange(len(sb)) - np.repeat(boff[:-1], bc)
        pos = rank + 1
        # per-(bucket, node-in-tile) counts -> inclusive cumsum = boundaries
        nk = sb * TN + (sd % TN)
        cntn = np.bincount(nk, minlength=NBKT * TN).reshape(NBKT, TN)
        cum = np.cumsum(cntn, axis=1)
        cores.append(dict(sb=sb, si=si, pos=pos, cum=cum))
        cnt_b = bc.reshape(NT, 2, NCORES)
        Wreq = np.maximum(Wreq, cnt_b.max(axis=2))

    Ws = tuple(
        tuple(int(32 * ((1 + Wreq[t, h] + 31) // 32)) for h in range(2))
        for t in range(NT)
    )
    assert max(w for tw in Ws for w in tw) <= 4096, Ws
    SM = sum(w // 16 for tw in Ws for w in tw)
    soff_flat = np.zeros(NBKT, np.int64)   # idx col offset per bucket
    acc = 0
    for t in range(NT):
        for h in range(2):
            for g in range(NCORES):
                soff_flat[(t * 2 + h) * NCORES + g] = acc
            acc += Ws[t][h] // 16
    SBT = NT * 2 * SBH

    if Ws not in _NEFF_CACHE:
        _NEFF_CACHE[Ws] = build_neff(Ws)
    nc = _NEFF_CACHE[Ws]

    # ---- per-core input maps ---------------------------------------------
    w1t_in = np.ascontiguousarray(
        W1.T.reshape(4, 128, HID).transpose(1, 0, 2)
    ).astype(NPBF)
    w2b_in = np.concatenate(
        [W2.T.astype(np.float32), b2.reshape(1, CLS).astype(np.float32)], axis=0
    )
    g8_in = np.zeros((128, 16), np.float32)
    g8_in[np.arange(128), np.arange(128) % 16] = 1.0

    bnd_i = np.arange(NB)
    bnd_part16 = (bnd_i % 16).astype(np.int64)
    bnd_col = (bnd_i // 16).astype(np.int64)

    # pre-concatenated global input arrays ([NCORES*d0, ...])
    glob = dict(
        xT=np.empty((NCORES * F_IN, NPAD), NPF8),
        w1t=np.broadcast_to(
            w1t_in, (NCORES, 128, 4, HID)
        ).reshape(NCORES * 128, 4, HID),
        b1r=np.broadcast_to(
            b1.reshape(1, HID).astype(NPBF), (NCORES, HID)
        ).reshape(NCORES * 1, HID),
        dvr=np.empty((NCORES * NCH, CH), NPBF),
        dv2=np.empty((NCORES * NCH, CH), np.float32),
        rvr=np.empty((NCORES * NCH, CH), np.float32),
        dsc=np.empty((NCORES * 128, NPAD // 128), np.float32),
        w2b=np.broadcast_to(
            w2b_in, (NCORES, HID + 1, CLS)
        ).reshape(NCORES * (HID + 1), CLS),
        g8m=np.broadcast_to(
            g8_in, (NCORES, 128, 16)
        ).reshape(NCORES * 128, 16),
        mix=np.zeros((NCORES * 128, SM), np.uint16),
        bix=np.zeros((NCORES * 128, SBT), np.uint16),
    )
    for c in range(NCORES):
        cc = cores[c]
        lo = c * NP
        dv_l = np.zeros(NPAD, np.float32)
        dv_l[:NP] = dinv[lo:lo + NP]
        rv_l = np.zeros(NPAD, np.float32)
        rv_l[:NP] = rvec[lo:lo + NP]
        xs = np.zeros((NPAD, F_IN), np.float32)
        xs[:NP] = x[lo:lo + NP] * dinv[lo:lo + NP, None]
        glob["xT"][c * F_IN:(c + 1) * F_IN] = xs.T.astype(NPF8)
        glob["dvr"][c * NCH:(c + 1) * NCH] = dv_l.reshape(NCH, CH).astype(NPBF)
        glob["dv2"][c * NCH:(c + 1) * NCH] = (dv_l * dv_l).reshape(NCH, CH)
        glob["rvr"][c * NCH:(c + 1) * NCH] = rv_l.reshape(NCH, CH)
        glob["dsc"][c * 128:(c + 1) * 128] = dv_l.reshape(NPAD // 128, 128).T

        part = 16 * (cc["sb"] % NCORES) + (cc["pos"] & 15)
        colx = soff_flat[cc["sb"]] + (cc["pos"] >> 4)
        glob["mix"][c * 128 + part, colx] = cc["si"]

        # boundary stream per (tau, h, g): [0, cum[0..TN-1]] padded to NB
        bnd = np.zeros((NBKT, NB), np.uint16)
        bnd[:, 1:TN + 1] = cc["cum"].astype(np.uint16)
        for t in range(NT):
            for h in range(2):
                base = (t * 2 + h) * SBH
                for g in range(NCORES):
                    b = (t * 2 + h) * NCORES + g
                    glob["bix"][
                        c * 128 + 16 * g + bnd_part16, base + bnd_col
                    ] = bnd[b]

    res = _run_neff(nc, glob)
    _EXEC_NS["total"] = sum(_EXEC_NS["walls"]) * 1e9
    _EXEC_NS["have"] = False   # no ntff hook; test.py falls back to walls

    out = np.empty((n, CLS), np.float32)
    for c in range(NCORES):
        out[c * NP:(c + 1) * NP] = res[c]["oout"][:NP].astype(np.float32)
    return out


def last_exec_time_ns():
    return _EXEC_NS["total"] if _EXEC_NS["have"] else None


def last_run_walls():
    return list(_EXEC_NS["walls"])
